# revision 10
# baseline (speedup 1.0000x reference)
"""Trainium2 Bass kernel for nn_GATTrafficPredictionModel.

Mathematical collapse exploited (holds for every input by construction of the
model, not by luck of the data):
  - h = broadcast(x[:, -1, :]) makes all N=512 node features identical per
    sample, and the adjacency is dense all-ones.
  - GAT attention scores e[i,j] = leakyrelu(s_src[i] + s_dst[j]) are therefore
    constant over (i, j), so softmax over neighbors is exactly uniform (1/512,
    exact in fp32 since 512 is a power of two), and the attention-weighted sum
    of identical rows reproduces the row itself.  Both GAT layers collapse to
    per-sample linear maps; a1/a2 attention vectors drop out entirely.

Collapsed computation (B=32, F=128, K=8, H=64, C=64, N=512):
    z      = x[:, -1, :]                          (B, F)
    u      = elu(z @ W_heads)  flattened heads    (B, K*H)
    w_row  = u @ W_out                            (B, C)
    S      = sum_n Wf.reshape(C, N, C)[:, n, :]   (C, C)
    out    = w_row @ S.T + bf                     (B, C)

Sharding: the only large input is Wf (64 x 32768 fp32, 8 MiB).  Each of the 8
cores owns 8 output channels c' (8 contiguous rows of Wf), reduces them to
S^T[:, c'_range] on-device, and computes its disjoint slice out^T[c'_range, :].
The tiny upstream GEMMs (u, w_row) are replicated on every core.

Dataflow (v2, PE-reduce):
  - Wf ships as float8e3 (e3m4, 4 mantissa bits) scaled by 1536 so values
    land inside e3m4's normal range; 262 KiB/core, half the fp16 traffic.
    Host layout [n_mid(128p), (n_out(4), c'(8), c2(64))] puts pure n on the
    partition axis, so the whole n-reduction runs on the (otherwise idle)
    PE as 32 accumulating matmuls against a ones column:
        st[c2, c'] += wf8[:, (no*8+c')*64 : +64]^T @ ones      (PSUM fp32)
    This yields S^T (x1536) directly in the layout the final matmul needs;
    the 1/1536 rides the final activation's scale.  The DVE (the previous
    bottleneck at ~2.3us of fp16 reduces) now only does three tiny copies.
  - The small fp16 pack (z^T | W_heads^T | W_out^T | bias bits) feeds the
    replicated GEMM chain: wh = Wh^T z (PE), elu via Relu/Exp activations
    (ACT) + one DVE add + ACT identity with bias=-1, wr = Wo^T u (PE),
    out^T = (S^T)^T wr / 1536 + bf (PE + ACT).
  - DMA queues: wf8 chunks issue from SP, small + output from ACT, so the
    per-DMA sequencer cost (~650ns) is split across the two HWDGE queues.

Precision: e3m4 Wf + fp16 smalls + fp32 PSUM accumulation everywhere.
Measured end-to-end relative error vs the fp32 jax reference: ~9.5e-3
(harness gate 2e-2; the pipeline is deterministic -- the fp8/fp16 casts
happen on host and the device accumulates in fp32, so the locally measured
error is exactly what the harness sees).  Per-core DMA traffic
468 KiB vs the baseline's 742 KiB.
"""

import os
import numpy as np
import ml_dtypes

import concourse.bass as bass
import concourse.bacc as bacc
import concourse.mybir as mybir
import concourse.tile as tile
from concourse.bass_utils import run_bass_kernel_spmd

N_CORES = 8
B, S_SEQ, F = 32, 12, 128
K, H, C, N = 8, 64, 64, 512
ROWS = C // N_CORES          # output channels per core
F32 = mybir.dt.float32
F16 = mybir.dt.float16
F8E3 = mybir.dt.float8e3
AF = mybir.ActivationFunctionType

WF_SCALE = 1536.0            # Wf -> e3m4 scale; 1/WF_SCALE folded into final ACT

# fp16 small-pack column layout: zt | wht | wot | aux-bits
# (aux = the fp32 bias row bf[c'0:c'0+8] carried as raw bits in 16 fp16 cols
#  of partition 0; a bitcast view recovers the [1, 8] fp32 AP)
ZT0 = 0
WHT0 = ZT0 + B
WOT0 = WHT0 + K * H
AUXBITS0 = WOT0 + 4 * C      # must be even (fp32 view needs 4B alignment)
SMALL_COLS = AUXBITS0 + 2 * ROWS

# wf8 chunking: 2 chunks of 2 n_out blocks each overlap DMA with PE reduce
WF_CHUNKS = int(os.environ.get("KV_WF_CHUNKS", "2"))
assert 4 % WF_CHUNKS == 0
NO_PER_CHUNK = 4 // WF_CHUNKS


def _emit_body(nc, pool, wfpool, psum, t, consts):
    """One full per-core computation; `t` maps dram tensor names to handles.

    Queue discipline (this is what makes iterations pipeline): every INPUT
    DMA issues from SP, whose stream is never gated on compute (only on
    tile-pool buffer reuse, which has >=1 iteration of slack).  ACT runs
    pure compute.  The output DMA -- the one issue that IS gated on the
    whole iteration's compute -- goes to the otherwise idle GPSIMD SWDGE
    queue so it cannot delay any next-iteration input issue.
    """
    ones8 = consts["ones8"]

    wf_tiles = []
    for ci in range(WF_CHUNKS):
        w = 512 * NO_PER_CHUNK
        tl = wfpool.tile([128, w], F8E3, tag=f"wf{ci}", name=f"wf{ci}")
        nc.sync.dma_start(tl[:], t["wf8"][:, ci * w:(ci + 1) * w])
        wf_tiles.append(tl)
    small_s = pool.tile([128, SMALL_COLS], F16, tag="small")
    nc.sync.dma_start(small_s[:], t["small"][:])

    zt_s = small_s[:, ZT0:ZT0 + B]
    wht_s = small_s[:, WHT0:WHT0 + K * H]
    wot_s = small_s[:, WOT0:WOT0 + 4 * C]
    bft_row = small_s[0:1, AUXBITS0:AUXBITS0 + 2 * ROWS].bitcast(F32)

    # ---- Wf shard -> S^T[:, core slice] (x WF_SCALE), all on PE ----------
    # wf8[p, f]: p = n_mid, f = (n_out, c', c2).  Each 64-col block is one
    # (n_out, c') slab; summing its partitions (matmul vs ones) gives the
    # n-partial of S^T[:, c'], accumulated across n_out in PSUM.
    st_p = psum.tile([C, ROWS], F32, tag="stp")
    for ci in range(WF_CHUNKS):
        tl = wf_tiles[ci]
        for no_l in range(NO_PER_CHUNK):
            no = ci * NO_PER_CHUNK + no_l
            for cp in range(ROWS):
                blk = tl[:, (no_l * ROWS + cp) * C:(no_l * ROWS + cp) * C + C]
                # one accumulation group for the whole tile: the first
                # matmul's start marks the 2KB zero region pending-zero, so
                # each column's first write lands on zeros and later writes
                # accumulate (PSUM start/stop is region-, not AP-, scoped)
                nc.tensor.matmul(
                    st_p[:, cp:cp + 1], blk, ones8[:],
                    start=(no == 0 and cp == 0),
                    stop=(no == 3 and cp == ROWS - 1),
                )
        if ci == 0:
            # ---- u^T = elu(W_heads^T z) / WF_SCALE, emitted between chunks
            # so the PE works on it while the last wf chunk streams --------
            wh_p = psum.tile([128, 4 * B], F32, tag="whp")
            for j in range(4):
                nc.tensor.matmul(
                    wh_p[:, B * j:B * (j + 1)],
                    wht_s[:, 128 * j:128 * (j + 1)],
                    zt_s,
                    start=True, stop=True,
                )
            # elu(x) = relu(x) + exp(-relu(-x)) - 1; the final identity
            # folds the 1/WF_SCALE (cancelling the Wf fp8 scale) into both
            # scale and bias, so no downstream op pays for it.
            rneg_t = pool.tile([128, 4 * B], F16, tag="rneg")
            nc.scalar.activation(rneg_t[:], wh_p[:], AF.Relu, scale=-1.0)
            exp_t = pool.tile([128, 4 * B], F16, tag="exp")
            nc.scalar.activation(exp_t[:], rneg_t[:], AF.Exp, scale=-1.0)
            relu_t = pool.tile([128, 4 * B], F16, tag="relu")
            nc.scalar.activation(relu_t[:], wh_p[:], AF.Relu)
            sum_t = pool.tile([128, 4 * B], F16, tag="sum")
            nc.vector.tensor_add(sum_t[:], relu_t[:], exp_t[:])
            u_t = pool.tile([128, 4 * B], F16, tag="u")
            nc.scalar.activation(u_t[:], sum_t[:], AF.Identity,
                                 scale=1.0 / WF_SCALE,
                                 bias=consts["neg_inv"][:])

    # ---- w_row^T / WF_SCALE = W_out^T u --------------------------------
    wr_p = psum.tile([C, B], F32, tag="wrp")
    for j in range(4):
        nc.tensor.matmul(
            wr_p[:], wot_s[:, C * j:C * (j + 1)], u_t[:, B * j:B * (j + 1)],
            start=(j == 0), stop=(j == 3),
        )
    wr_s = pool.tile([C, B], F32, tag="wrs")
    nc.vector.tensor_copy(wr_s[:], wr_p[:])
    st_s = pool.tile([C, ROWS], F32, tag="sts")
    nc.vector.tensor_copy(st_s[:], st_p[:])

    # ---- out^T[c' slice] = (S^T x SCALE)^T (w_row^T / SCALE) + bf --------
    # bf rides a 1-partition rank-1 matmul into the same PSUM accumulation,
    # so no activation sits between the PE and the output DMA.
    o_p = psum.tile([ROWS, B], F32, tag="op")
    nc.tensor.matmul(o_p[:], st_s[:], wr_s[:], start=True, stop=False)
    nc.tensor.matmul(o_p[:], bft_row, consts["ones_row"][:],
                     start=False, stop=True)
    o_s = pool.tile([ROWS, B], F32, tag="os")
    nc.vector.tensor_copy(o_s[:], o_p[:])
    nc.gpsimd.dma_start(t["out"][:], o_s[:])


def _build_nc(reps=1, loop_iters=None):
    nc = bacc.Bacc("TRN2", target_bir_lowering=False, debug=False,
                   num_devices=N_CORES)

    t = {
        "wf8": nc.dram_tensor("wf8", [128, 2048], F8E3, kind="ExternalInput"),
        "small": nc.dram_tensor("small", [128, SMALL_COLS], F16,
                                kind="ExternalInput"),
        "out": nc.dram_tensor("out", [ROWS, B], F32, kind="ExternalOutput"),
    }

    with tile.TileContext(nc) as tc:
        with (
            tc.tile_pool(name="consts", bufs=1) as const_pool,
            tc.tile_pool(name="pool", bufs=3) as pool,
            tc.tile_pool(name="wfpool", bufs=2) as wfpool,
            tc.tile_pool(name="psum", bufs=2, space=bass.MemorySpace.PSUM) as psum,
        ):
            ones8 = const_pool.tile([128, 1], F8E3, tag="ones8")
            nc.vector.memset(ones8[:], 1.0)
            neg_inv = const_pool.tile([128, 1], F32, tag="neg_inv")
            nc.vector.memset(neg_inv[:], -1.0 / WF_SCALE)
            ones_row = const_pool.tile([1, B], F32, tag="ones_row")
            nc.vector.memset(ones_row[:], 1.0)
            consts = {"ones8": ones8, "neg_inv": neg_inv,
                      "ones_row": ones_row}
            if loop_iters:
                tc.For_i_unrolled(
                    0, loop_iters, 1,
                    lambda iv: _emit_body(nc, pool, wfpool, psum, t, consts),
                    max_unroll=8,
                )
            else:
                for _rep in range(reps):
                    _emit_body(nc, pool, wfpool, psum, t, consts)

    nc.compile()
    return nc


_NC_CACHE = None
_last_in_maps = None


def _make_in_maps(x, W_heads, W_out, Wf, bf):
    x = np.ascontiguousarray(np.asarray(x, np.float32))
    W_heads = np.ascontiguousarray(np.asarray(W_heads, np.float32))
    W_out = np.ascontiguousarray(np.asarray(W_out, np.float32))
    Wf = np.ascontiguousarray(np.asarray(Wf, np.float32))
    bf = np.ascontiguousarray(np.asarray(bf, np.float32))

    small = np.zeros((128, SMALL_COLS), np.float16)
    small[:, ZT0:ZT0 + B] = x[:, -1, :].T                          # (128, 32)
    small[:, WHT0:WHT0 + K * H] = \
        W_heads.transpose(1, 0, 2).reshape(F, K * H)               # (128, 512)
    small[:, WOT0:WOT0 + 4 * C] = \
        W_out.reshape(4, 128, C).transpose(1, 0, 2).reshape(128, 4 * C)

    in_maps = []
    for c in range(N_CORES):
        # wf8: [c'(8), n(512), c2(64)] -> [n_mid(128), (n_out(4), c'(8), c2)]
        shard = Wf[ROWS * c:ROWS * (c + 1)].reshape(ROWS, 4, 128, C)
        wf_host = np.ascontiguousarray(
            (shard.transpose(2, 1, 0, 3) * WF_SCALE)
            .astype(ml_dtypes.float8_e3m4)
        ).reshape(128, 2048)
        small_c = small.copy()
        small_c[0, AUXBITS0:AUXBITS0 + 2 * ROWS] = \
            np.ascontiguousarray(bf[ROWS * c:ROWS * (c + 1)]).view(np.float16)
        in_maps.append({"wf8": wf_host, "small": small_c})
    return in_maps


def kernel(x, W_heads, a1_heads, a2_heads, W_out, a1_out, a2_out, Wf, bf):
    global _NC_CACHE
    if _NC_CACHE is None:
        _NC_CACHE = _build_nc()
    nc = _NC_CACHE

    in_maps = _make_in_maps(x, W_heads, W_out, Wf, bf)
    global _last_in_maps
    _last_in_maps = in_maps
    res = run_bass_kernel_spmd(nc, in_maps, list(range(N_CORES)))
    outT = np.concatenate([res.results[i]["out"] for i in range(N_CORES)], axis=0)
    return np.ascontiguousarray(outT.T)                            # (32, 64)


# revision 11
# speedup vs baseline: 12.3179x; 12.3179x over previous
"""Trainium2 Bass kernel for nn_GATTrafficPredictionModel.

Mathematical collapse exploited (holds for every input by construction of the
model, not by luck of the data):
  - h = broadcast(x[:, -1, :]) makes all N=512 node features identical per
    sample, and the adjacency is dense all-ones.
  - GAT attention scores e[i,j] = leakyrelu(s_src[i] + s_dst[j]) are therefore
    constant over (i, j), so softmax over neighbors is exactly uniform (1/512,
    exact in fp32 since 512 is a power of two), and the attention-weighted sum
    of identical rows reproduces the row itself.  Both GAT layers collapse to
    per-sample linear maps; a1/a2 attention vectors drop out entirely.

Collapsed computation (B=32, F=128, K=8, H=64, C=64, N=512):
    z      = x[:, -1, :]                          (B, F)
    u      = elu(z @ W_heads)  flattened heads    (B, K*H)
    w_row  = u @ W_out                            (B, C)
    S      = sum_n Wf.reshape(C, N, C)[:, n, :]   (C, C)
    out    = w_row @ S.T + bf                     (B, C)

Sharding: each of the 8 cores owns 8 output channels c' (8 contiguous rows
of Wf, the only large input), reduces them to S^T[:, c'_range] on-device and
computes its disjoint slice out^T[c'_range, :].  The tiny upstream GEMMs are
replicated on every core.

Dataflow (v3, single-DMA + PE-reduce).  Measured reality on this part
(microbenchmarks, see session notes): each dma_start costs ~650ns of
sequencer time on its queue and a solo DMA ~1.3us (SEQ+HWDGE serialize),
while PE matmuls including their unmodeled weight loads are effectively
free at this scale.  So the whole iteration is built around ONE input DMA:

  - Everything ships in one [128, 3680] float8e3 tensor: cols 0..2047 are
    Wf x 1536 quantized to e3m4 (4 mantissa bits), the rest carries the
    fp16 small pack (z^T | W_heads^T | W_out^T | fp32 bias bits) as raw
    bytes, recovered on-device with bitcast views.  469 KiB/core total,
    ~1.32us at the 358 GB/s HBM roofline (vs 742 KiB for the all-fp16
    baseline).
  - Wf host layout [n_mid(128p), (n_out(4), c'(8), c2(64))] puts pure n on
    the partition axis so the whole n-reduction runs on the otherwise-idle
    PE as 32 accumulating matmuls against a ones column, yielding S^T
    (x1536) directly in the layout the final matmul wants.  One PSUM
    accumulation group for the tile: start only on the first matmul (PSUM
    start/stop is 2KB-zero-region-, not AP-, scoped).
  - elu(x)/1536 = relu(x)/1536 + exp(min(x,0))/1536 - 1/1536 splits across
    engines: min and scaled-relu on DVE (tensor_scalar), exp on ACT with
    the 1/1536 folded into its bias as ln(1/1536), the -1/1536 via an ACT
    identity bias.  The 1536 cancels against the Wf quantization scale in
    the final matmul; the bias bf rides a rank-1 single-partition matmul
    into the same output PSUM group, so nothing sits between PE and the
    output DMA but one DVE copy.
  - Queues: the input DMA issues from SP, the output from the GPSIMD SWDGE
    queue; ACT only computes.  No queue carries two compute-gated issues,
    so iterations pipeline at the transfer roofline.

Precision: e3m4 Wf + fp16 smalls + fp32 PSUM accumulation everywhere;
end-to-end relative error vs the fp32 jax reference: 9.45e-3 (harness gate
2e-2).  The pipeline is deterministic: the fp8/fp16 casts happen on host
and the device accumulates in fp32, so the locally measured error is
exactly what the harness sees (numpy model matches device to ~1e-5).
"""

import os
import numpy as np
import ml_dtypes

import concourse.bass as bass
import concourse.bacc as bacc
import concourse.mybir as mybir
import concourse.tile as tile
from concourse.bass_utils import run_bass_kernel_spmd

N_CORES = 8
B, S_SEQ, F = 32, 12, 128
K, H, C, N = 8, 64, 64, 512
ROWS = C // N_CORES          # output channels per core
F32 = mybir.dt.float32
F16 = mybir.dt.float16
F8E3 = mybir.dt.float8e3
AF = mybir.ActivationFunctionType

WF_SCALE = 1536.0            # Wf -> e3m4 scale; cancelled via the elu scale

# pack layout (fp8 cols): [wf8 2048 | fp16 smalls as 2 cols each]
# fp16 small-pack sublayout (fp16 cols): zt 32 | wht 512 | wot 256 | aux 16
ZT0 = 0
WHT0 = ZT0 + B
WOT0 = WHT0 + K * H
AUXBITS0 = WOT0 + 4 * C      # 16 fp16 cols = [1, 8] fp32 bias row bits
SMALL16 = AUXBITS0 + 2 * ROWS
WF_COLS = 2048
PACK_COLS = WF_COLS + 2 * SMALL16

# number of dma_starts the pack is split into (1 = single DMA)
PACK_CHUNKS = int(os.environ.get("KV_PACK_CHUNKS", "1"))


def _emit_body(nc, pool, psum, t, consts):
    """One full per-core computation; `t` maps dram tensor names to handles.

    Queue discipline (this is what makes iterations pipeline): the input
    DMA issues from SP, whose stream is never gated on compute.  ACT and
    DVE run pure compute.  The output DMA -- the one issue that IS gated on
    the iteration's compute -- goes to the otherwise idle GPSIMD SWDGE
    queue so it cannot delay any next-iteration input issue.
    """
    pack_s = pool.tile([128, PACK_COLS], F8E3, tag="pack")
    if PACK_CHUNKS == 1:
        nc.sync.dma_start(pack_s[:], t["pack"][:])
    else:
        w = PACK_COLS // PACK_CHUNKS
        for ci in range(PACK_CHUNKS):
            lo, hi = ci * w, (ci + 1) * w if ci < PACK_CHUNKS - 1 else PACK_COLS
            nc.sync.dma_start(pack_s[:, lo:hi], t["pack"][:, lo:hi])

    wf_s = pack_s[:, 0:WF_COLS]
    sm = pack_s[:, WF_COLS:WF_COLS + 2 * SMALL16].bitcast(F16)
    zt_s = sm[:, ZT0:ZT0 + B]
    wht_s = sm[:, WHT0:WHT0 + K * H]
    wot_s = sm[:, WOT0:WOT0 + 4 * C]
    bft_row = sm[0:1, AUXBITS0:AUXBITS0 + 2 * ROWS].bitcast(F32)

    # ---- Wf shard -> S^T[:, core slice] (x WF_SCALE), all on PE ----------
    # wf8[p, f]: p = n_mid, f = (n_out, c', c2).  Each 64-col block is one
    # (n_out, c') slab; summing its partitions (matmul vs a ones column)
    # gives the n-partial of S^T[:, c'], accumulated across n_out in PSUM.
    st_p = psum.tile([C, ROWS], F32, tag="stp")
    for no in range(4):
        for cp in range(ROWS):
            blk = wf_s[:, (no * ROWS + cp) * C:(no * ROWS + cp) * C + C]
            nc.tensor.matmul(
                st_p[:, cp:cp + 1], blk, consts["ones8"][:],
                start=(no == 0 and cp == 0),
                stop=(no == 3 and cp == ROWS - 1),
            )

    # ---- u^T = elu(W_heads^T z) / WF_SCALE ------------------------------
    wh_p = psum.tile([128, 4 * B], F32, tag="whp")
    for j in range(4):
        nc.tensor.matmul(
            wh_p[:, B * j:B * (j + 1)],
            wht_s[:, 128 * j:128 * (j + 1)],
            zt_s,
            start=True, stop=True,
        )
    # elu(x)/s = relu(x)/s + exp(min(x,0))/s - 1/s, engines split so no one
    # engine owns the chain: DVE does min and scaled relu, ACT does exp
    # (1/s folded into its bias: e^(x + ln(1/s)) = e^x/s) and the -1/s.
    min_t = pool.tile([128, 4 * B], F16, tag="min")
    nc.vector.tensor_scalar_min(min_t[:], wh_p[:], 0.0)
    exp_t = pool.tile([128, 4 * B], F16, tag="exp")
    nc.scalar.activation(exp_t[:], min_t[:], AF.Exp, bias=consts["lninv"][:])
    rel_t = pool.tile([128, 4 * B], F16, tag="rel")
    nc.vector.tensor_scalar(rel_t[:], wh_p[:], 0.0, 1.0 / WF_SCALE,
                            mybir.AluOpType.max, mybir.AluOpType.mult)
    sum_t = pool.tile([128, 4 * B], F16, tag="sum")
    nc.vector.tensor_add(sum_t[:], rel_t[:], exp_t[:])
    u_t = pool.tile([128, 4 * B], F16, tag="u")
    nc.scalar.activation(u_t[:], sum_t[:], AF.Identity,
                         bias=consts["neg_inv"][:])

    # ---- w_row^T / WF_SCALE = W_out^T u ---------------------------------
    wr_p = psum.tile([C, B], F32, tag="wrp")
    for j in range(4):
        nc.tensor.matmul(
            wr_p[:], wot_s[:, C * j:C * (j + 1)], u_t[:, B * j:B * (j + 1)],
            start=(j == 0), stop=(j == 3),
        )
    wr_s = pool.tile([C, B], F32, tag="wrs")
    nc.vector.tensor_copy(wr_s[:], wr_p[:])
    st_s = pool.tile([C, ROWS], F32, tag="sts")
    nc.vector.tensor_copy(st_s[:], st_p[:])

    # ---- out^T[c' slice] = (S^T x s)^T (w_row^T / s) + bf ----------------
    # bf rides a 1-partition rank-1 matmul into the same PSUM accumulation,
    # so nothing but one DVE copy sits between the PE and the output DMA.
    o_p = psum.tile([ROWS, B], F32, tag="op")
    nc.tensor.matmul(o_p[:], st_s[:], wr_s[:], start=True, stop=False)
    nc.tensor.matmul(o_p[:], bft_row, consts["ones_row"][:],
                     start=False, stop=True)
    o_s = pool.tile([ROWS, B], F32, tag="os")
    nc.vector.tensor_copy(o_s[:], o_p[:])
    nc.gpsimd.dma_start(t["out"][:], o_s[:])


def _build_nc(reps=1, loop_iters=None):
    nc = bacc.Bacc("TRN2", target_bir_lowering=False, debug=False,
                   num_devices=N_CORES)

    t = {
        "pack": nc.dram_tensor("pack", [128, PACK_COLS], F8E3,
                               kind="ExternalInput"),
        "out": nc.dram_tensor("out", [ROWS, B], F32, kind="ExternalOutput"),
    }

    with tile.TileContext(nc) as tc:
        with (
            tc.tile_pool(name="consts", bufs=1) as const_pool,
            tc.tile_pool(name="pool", bufs=3) as pool,
            tc.tile_pool(name="psum", bufs=2, space=bass.MemorySpace.PSUM) as psum,
        ):
            ones8 = const_pool.tile([128, 1], F8E3, tag="ones8")
            nc.vector.memset(ones8[:], 1.0)
            neg_inv = const_pool.tile([128, 1], F32, tag="neg_inv")
            nc.vector.memset(neg_inv[:], -1.0 / WF_SCALE)
            lninv = const_pool.tile([128, 1], F32, tag="lninv")
            nc.vector.memset(lninv[:], float(-np.log(WF_SCALE)))
            ones_row = const_pool.tile([1, B], F32, tag="ones_row")
            nc.vector.memset(ones_row[:], 1.0)
            consts = {"ones8": ones8, "neg_inv": neg_inv, "lninv": lninv,
                      "ones_row": ones_row}
            if loop_iters:
                tc.For_i_unrolled(
                    0, loop_iters, 1,
                    lambda iv: _emit_body(nc, pool, psum, t, consts),
                    max_unroll=8,
                )
            else:
                for _rep in range(reps):
                    _emit_body(nc, pool, psum, t, consts)

    nc.compile()
    return nc


_NC_CACHE = None
_last_in_maps = None


def _make_in_maps(x, W_heads, W_out, Wf, bf):
    x = np.ascontiguousarray(np.asarray(x, np.float32))
    W_heads = np.ascontiguousarray(np.asarray(W_heads, np.float32))
    W_out = np.ascontiguousarray(np.asarray(W_out, np.float32))
    Wf = np.ascontiguousarray(np.asarray(Wf, np.float32))
    bf = np.ascontiguousarray(np.asarray(bf, np.float32))

    small = np.zeros((128, SMALL16), np.float16)
    small[:, ZT0:ZT0 + B] = x[:, -1, :].T                          # (128, 32)
    small[:, WHT0:WHT0 + K * H] = \
        W_heads.transpose(1, 0, 2).reshape(F, K * H)               # (128, 512)
    small[:, WOT0:WOT0 + 4 * C] = \
        W_out.reshape(4, 128, C).transpose(1, 0, 2).reshape(128, 4 * C)

    in_maps = []
    for c in range(N_CORES):
        # wf8: [c'(8), n(512), c2(64)] -> [n_mid(128), (n_out(4), c'(8), c2)]
        shard = Wf[ROWS * c:ROWS * (c + 1)].reshape(ROWS, 4, 128, C)
        wf8 = np.ascontiguousarray(
            (shard.transpose(2, 1, 0, 3) * WF_SCALE)
            .astype(ml_dtypes.float8_e3m4)
        ).reshape(128, WF_COLS)
        small_c = small.copy()
        small_c[0, AUXBITS0:AUXBITS0 + 2 * ROWS] = \
            np.ascontiguousarray(bf[ROWS * c:ROWS * (c + 1)]).view(np.float16)
        pack = np.empty((128, PACK_COLS), dtype=ml_dtypes.float8_e3m4)
        pb = pack.view(np.uint8)
        pb[:, 0:WF_COLS] = wf8.view(np.uint8)
        pb[:, WF_COLS:PACK_COLS] = small_c.view(np.uint8)
        in_maps.append({"pack": pack})
    return in_maps


def kernel(x, W_heads, a1_heads, a2_heads, W_out, a1_out, a2_out, Wf, bf):
    global _NC_CACHE
    if _NC_CACHE is None:
        _NC_CACHE = _build_nc()
    nc = _NC_CACHE

    in_maps = _make_in_maps(x, W_heads, W_out, Wf, bf)
    global _last_in_maps
    _last_in_maps = in_maps
    res = run_bass_kernel_spmd(nc, in_maps, list(range(N_CORES)))
    outT = np.concatenate([res.results[i]["out"] for i in range(N_CORES)], axis=0)
    return np.ascontiguousarray(outT.T)                            # (32, 64)


# revision 16
# speedup vs baseline: 20.1580x; 1.6365x over previous
"""Trainium2 Bass kernel for nn_GATTrafficPredictionModel.

Mathematical collapse exploited (holds for every input by construction of the
model, not by luck of the data):
  - h = broadcast(x[:, -1, :]) makes all N=512 node features identical per
    sample, and the adjacency is dense all-ones.
  - GAT attention scores e[i,j] = leakyrelu(s_src[i] + s_dst[j]) are therefore
    constant over (i, j), so softmax over neighbors is exactly uniform (1/512,
    exact in fp32 since 512 is a power of two), and the attention-weighted sum
    of identical rows reproduces the row itself.  Both GAT layers collapse to
    per-sample linear maps; a1/a2 attention vectors drop out entirely.

Collapsed computation (B=32, F=128, K=8, H=64, C=64, N=512):
    z      = x[:, -1, :]                          (B, F)
    u      = elu(z @ W_heads)  flattened heads    (B, K*H)
    w_row  = u @ W_out                            (B, C)
    S      = sum_n Wf.reshape(C, N, C)[:, n, :]   (C, C)
    out    = w_row @ S.T + bf                     (B, C)

Sharding: each of the 8 cores owns 8 output channels c' (8 contiguous rows
of Wf, the only large input), reduces them to S^T[:, c'_range] on-device and
computes its disjoint slice out^T[c'_range, :].  The tiny upstream GEMMs are
replicated on every core; host-side work is layout/quantization and
concatenating the eight disjoint (8, 32) output slices.

Dataflow (fp8-Wf + PE-reduce):
  - Wf ships as float8e3 (e3m4, 4 mantissa bits) scaled by 1536 into e3m4's
    normal range: 262 KiB/core, half the fp16 bytes; 469 KiB/core total
    traffic vs 742 KiB for the all-fp16 baseline.
  - Host layout [n_mid(128p), (n_out(4), c'(8), c2(64))] puts pure n on the
    partition axis, so the whole n-reduction runs on the PE as 32
    accumulating matmuls against a ones column (one PSUM group: start only
    on the first matmul -- PSUM start/stop is 2KB-zero-region-scoped).
    This frees the DVE, the baseline's bottleneck engine (~2.3us of fp16
    reduces), and yields S^T x1536 directly in the layout the final matmul
    wants.  The wf chunk tiles are consumed immediately by the PE, keeping
    their pool hold-windows short so iterations pipeline.
  - elu(x)/1536 = relu(x)/1536 + exp(min(x,0))/1536 - 1/1536 splits across
    engines: min and scaled-relu on DVE (tensor_scalar), exp on ACT with
    1/1536 folded into its bias as ln(1/1536), the -1/1536 via an ACT
    identity bias.  The 1536 cancels against the Wf quantization scale in
    the final matmul; the bias bf rides a rank-1 single-partition matmul
    into the output PSUM accumulation, so nothing but one DVE copy sits
    between the PE and the output DMA.
  - DMA structure mirrors the measured-fastest shape on this part: small
    pack first, then the two wf chunks, output last, all issued from SP
    (multiple medium DMAs pipeline better than one large one).

Precision: e3m4 Wf + fp16 smalls + fp32 PSUM accumulation everywhere;
end-to-end relative error vs the fp32 jax reference: 9.45e-3 (harness gate
2e-2).  The pipeline is deterministic: the fp8/fp16 casts happen on host
and the device only accumulates in fp32, so the locally measured error is
exactly what the harness sees (numpy model matches device to ~1e-5).
"""

import os
import numpy as np
import ml_dtypes

import concourse.bass as bass
import concourse.bacc as bacc
import concourse.mybir as mybir
import concourse.tile as tile
from concourse.bass_utils import run_bass_kernel_spmd

N_CORES = 8
B, S_SEQ, F = 32, 12, 128
K, H, C, N = 8, 64, 64, 512
ROWS = C // N_CORES
F32 = mybir.dt.float32
F16 = mybir.dt.float16
F8E3 = mybir.dt.float8e3
AF = mybir.ActivationFunctionType

WF_SCALE = 1536.0

ZT0 = 0
WHT0 = ZT0 + B
WOT0 = WHT0 + K * H
AUXBITS0 = WOT0 + 4 * C
SMALL16 = AUXBITS0 + 2 * ROWS
WF_COLS = 2048

WF_CHUNKS = int(os.environ.get("KV_WF_CHUNKS", "2"))
OUT_Q = os.environ.get("KV_OUT_Q", "sync")      # sync | scalar | gpsimd
SMALL_Q = os.environ.get("KV_SMALL_Q", "sync")  # sync | scalar
ORDER = os.environ.get("KV_ORDER", "small_first")  # wf_first | small_first
NO_PER_CHUNK = 4 // WF_CHUNKS


def _emit_body(nc, wfpool, pool, psum, t, consts):
    w = 512 * NO_PER_CHUNK
    small_s = pool.tile([128, SMALL16], F16, tag="small")
    small_q = getattr(nc, SMALL_Q)
    if ORDER == "small_first":
        small_q.dma_start(small_s[:], t["small"][:])
    wf_tiles = []
    tl = wfpool.tile([128, w], F8E3, tag="wf0", name="wf0")
    nc.sync.dma_start(tl[:], t["wf8"][:, 0:w])
    wf_tiles.append(tl)
    if ORDER != "small_first":
        small_q.dma_start(small_s[:], t["small"][:])
    for ci in range(1, WF_CHUNKS):
        tl = wfpool.tile([128, w], F8E3, tag=f"wf{ci}", name=f"wf{ci}")
        nc.sync.dma_start(tl[:], t["wf8"][:, ci * w:(ci + 1) * w])
        wf_tiles.append(tl)

    zt_s = small_s[:, ZT0:ZT0 + B]
    wht_s = small_s[:, WHT0:WHT0 + K * H]
    wot_s = small_s[:, WOT0:WOT0 + 4 * C]
    bft_row = small_s[0:1, AUXBITS0:AUXBITS0 + 2 * ROWS].bitcast(F32)

    st_p = psum.tile([C, ROWS], F32, tag="stp")
    for ci in range(WF_CHUNKS):
        tl = wf_tiles[ci]
        for no_l in range(NO_PER_CHUNK):
            no = ci * NO_PER_CHUNK + no_l
            for cp in range(ROWS):
                blk = tl[:, (no_l * ROWS + cp) * C:(no_l * ROWS + cp) * C + C]
                nc.tensor.matmul(
                    st_p[:, cp:cp + 1], blk, consts["ones8"][:],
                    start=(no == 0 and cp == 0),
                    stop=(no == 3 and cp == ROWS - 1),
                )
        if ci == 0:
            wh_p = psum.tile([128, 4 * B], F32, tag="whp")
            for j in range(4):
                nc.tensor.matmul(
                    wh_p[:, B * j:B * (j + 1)],
                    wht_s[:, 128 * j:128 * (j + 1)],
                    zt_s,
                    start=True, stop=True,
                )
            min_t = pool.tile([128, 4 * B], F16, tag="min")
            nc.vector.tensor_scalar_min(min_t[:], wh_p[:], 0.0)
            exp_t = pool.tile([128, 4 * B], F16, tag="exp")
            nc.scalar.activation(exp_t[:], min_t[:], AF.Exp,
                                 bias=consts["lninv"][:])
            rel_t = pool.tile([128, 4 * B], F16, tag="rel")
            nc.vector.tensor_scalar(rel_t[:], wh_p[:], 0.0, 1.0 / WF_SCALE,
                                    mybir.AluOpType.max, mybir.AluOpType.mult)
            sum_t = pool.tile([128, 4 * B], F16, tag="sum")
            nc.vector.tensor_add(sum_t[:], rel_t[:], exp_t[:])
            u_t = pool.tile([128, 4 * B], F16, tag="u")
            nc.scalar.activation(u_t[:], sum_t[:], AF.Identity,
                                 bias=consts["neg_inv"][:])

    wr_p = psum.tile([C, B], F32, tag="wrp")
    for j in range(4):
        nc.tensor.matmul(
            wr_p[:], wot_s[:, C * j:C * (j + 1)], u_t[:, B * j:B * (j + 1)],
            start=(j == 0), stop=(j == 3),
        )
    wr_s = pool.tile([C, B], F32, tag="wrs")
    nc.vector.tensor_copy(wr_s[:], wr_p[:])
    st_s = pool.tile([C, ROWS], F32, tag="sts")
    nc.vector.tensor_copy(st_s[:], st_p[:])

    o_p = psum.tile([ROWS, B], F32, tag="op")
    nc.tensor.matmul(o_p[:], st_s[:], wr_s[:], start=True, stop=False)
    nc.tensor.matmul(o_p[:], bft_row, consts["ones_row"][:],
                     start=False, stop=True)
    o_s = pool.tile([ROWS, B], F32, tag="os")
    nc.vector.tensor_copy(o_s[:], o_p[:])
    getattr(nc, OUT_Q).dma_start(t["out"][:], o_s[:])


def _build_nc(reps=1, loop_iters=None):
    nc = bacc.Bacc("TRN2", target_bir_lowering=False, debug=False,
                   num_devices=N_CORES)

    t = {
        "wf8": nc.dram_tensor("wf8", [128, WF_COLS], F8E3,
                              kind="ExternalInput"),
        "small": nc.dram_tensor("small", [128, SMALL16], F16,
                                kind="ExternalInput"),
        "out": nc.dram_tensor("out", [ROWS, B], F32, kind="ExternalOutput"),
    }

    with tile.TileContext(nc) as tc:
        with (
            tc.tile_pool(name="consts", bufs=1) as const_pool,
            tc.tile_pool(name="wfpool",
                         bufs=int(os.environ.get("KV_WF_BUFS", "2"))) as wfpool,
            tc.tile_pool(name="pool",
                         bufs=int(os.environ.get("KV_POOL_BUFS", "3"))) as pool,
            tc.tile_pool(name="psum", bufs=2, space=bass.MemorySpace.PSUM) as psum,
        ):
            ones8 = const_pool.tile([128, 1], F8E3, tag="ones8")
            nc.vector.memset(ones8[:], 1.0)
            neg_inv = const_pool.tile([128, 1], F32, tag="neg_inv")
            nc.vector.memset(neg_inv[:], -1.0 / WF_SCALE)
            lninv = const_pool.tile([128, 1], F32, tag="lninv")
            nc.vector.memset(lninv[:], float(-np.log(WF_SCALE)))
            ones_row = const_pool.tile([1, B], F32, tag="ones_row")
            nc.vector.memset(ones_row[:], 1.0)
            consts = {"ones8": ones8, "neg_inv": neg_inv, "lninv": lninv,
                      "ones_row": ones_row}
            if loop_iters:
                tc.For_i_unrolled(
                    0, loop_iters, 1,
                    lambda iv: _emit_body(nc, wfpool, pool, psum, t, consts),
                    max_unroll=8,
                )
            else:
                for _rep in range(reps):
                    _emit_body(nc, wfpool, pool, psum, t, consts)

    nc.compile()
    return nc


_NC_CACHE = None
_last_in_maps = None


def _make_in_maps(x, W_heads, W_out, Wf, bf):
    x = np.ascontiguousarray(np.asarray(x, np.float32))
    W_heads = np.ascontiguousarray(np.asarray(W_heads, np.float32))
    W_out = np.ascontiguousarray(np.asarray(W_out, np.float32))
    Wf = np.ascontiguousarray(np.asarray(Wf, np.float32))
    bf = np.ascontiguousarray(np.asarray(bf, np.float32))

    small = np.zeros((128, SMALL16), np.float16)
    small[:, ZT0:ZT0 + B] = x[:, -1, :].T
    small[:, WHT0:WHT0 + K * H] = \
        W_heads.transpose(1, 0, 2).reshape(F, K * H)
    small[:, WOT0:WOT0 + 4 * C] = \
        W_out.reshape(4, 128, C).transpose(1, 0, 2).reshape(128, 4 * C)

    in_maps = []
    for c in range(N_CORES):
        shard = Wf[ROWS * c:ROWS * (c + 1)].reshape(ROWS, 4, 128, C)
        wf8 = np.ascontiguousarray(
            (shard.transpose(2, 1, 0, 3) * WF_SCALE)
            .astype(ml_dtypes.float8_e3m4)
        ).reshape(128, WF_COLS)
        small_c = small.copy()
        small_c[0, AUXBITS0:AUXBITS0 + 2 * ROWS] = \
            np.ascontiguousarray(bf[ROWS * c:ROWS * (c + 1)]).view(np.float16)
        in_maps.append({"wf8": wf8, "small": small_c})
    return in_maps


def kernel(x, W_heads, a1_heads, a2_heads, W_out, a1_out, a2_out, Wf, bf):
    global _NC_CACHE
    if _NC_CACHE is None:
        _NC_CACHE = _build_nc()
    nc = _NC_CACHE

    in_maps = _make_in_maps(x, W_heads, W_out, Wf, bf)
    global _last_in_maps
    _last_in_maps = in_maps
    res = run_bass_kernel_spmd(nc, in_maps, list(range(N_CORES)))
    outT = np.concatenate([res.results[i]["out"] for i in range(N_CORES)], axis=0)
    return np.ascontiguousarray(outT.T)
